# revision 1
# baseline (speedup 1.0000x reference)
"""GCN encoder (3x [GCNConv -> ReLU -> BatchNorm]) on 8 Trainium2 NeuronCores.

Strategy (graph/data parallel, dst-sharded):
  - Nodes sharded 8 ways by dst; each core owns its node shard and all edges
    whose dst lands in the shard (plus one self-loop edge per node).
  - Per layer: each core computes g = dinv * (x @ W) for its shard (PE),
    AllGather g -> full bf16 table in DRAM (rows padded to 256 B for the
    gather engine), then dma_gather pulls g[src] rows for its edges,
    the TensorEngine segment-sums them using on-the-fly one-hot matrices
    (DVE iota-compare), then dinv/bias/ReLU, BN statistics (free-axis
    reduce; feature-major layout), AllReduce of the 64x2 stats, BN affine.
  - Feature data is bf16; accumulation f32 in PSUM; statistics f32.

The gather uses the custom InstDMAGatherAnt (int16 indices, 256B-multiple
row stride), so sources are bucketed by table-row>>15 into 4 index buckets;
chunks of 128 edges are (window x bucket)-pure. Window = 64-dst grid for
the one-hot; chunk counts per (window, bucket) cell are equalized across
cores (SPMD: one program, per-core data).

Host-side preprocessing (numpy, off the measured HW path): degree/dinv,
edge sharding + cell packing, index tables, final unshard/transpose.
"""

import os
import numpy as np
import ml_dtypes

import concourse.bass as bass
import concourse.bacc as bacc
import concourse.mybir as mybir
import concourse.tile as tile
from concourse import bass_utils

BF16 = ml_dtypes.bfloat16
BROW = 128          # table row width (elements) -> 256 B in bf16
BUCK = 1 << 15      # index-bucket size (int16 positive range)

# ---------------------------------------------------------------- config

class Cfg:
    def __init__(self, N, E, D=64, ncores=8, eps=1e-5):
        self.N = N
        self.E = E
        self.D = D
        self.ncores = ncores
        self.eps = eps
        self.SH = N // ncores
        assert self.SH * ncores == N
        self.W = 64              # one-hot window width (dst grid)
        self.STW = 512           # dsts per supertile (one PSUM bank)
        self.SHP = ((self.SH + 511) // 512) * 512
        self.NST = self.SHP // self.STW
        self.NW = self.SHP // self.W
        self.GC = self.SHP // 128
        self.NTAB = ncores * self.SHP
        self.NBUCK = (self.NTAB + BUCK - 1) // BUCK


REAL = Cfg(N=100000, E=1600000)

# ---------------------------------------------------------------- host prep

def preprocess(cfg, edge_index):
    """Build per-core gather/segment tables + the shared compile-time plan.

    Chunk order: [supertile][bucket][window-within-st]; each chunk is 128
    edge slots, (window, bucket)-pure. Gather call (st, b) covers that
    bucket's contiguous chunk range of the supertile.

    Returns (dinv, plan, per_core) where plan has:
      chunk_win[k]   window-within-st (0..7) of chunk k
      st_ranges[st]  (k0, k1) chunk range of supertile st
      calls[st]      list of (k0, k1, bucket) gather calls
    """
    src = np.asarray(edge_index[0], dtype=np.int64)
    dst = np.asarray(edge_index[1], dtype=np.int64)
    N, SH, SHP, W, NW, nc_ = cfg.N, cfg.SH, cfg.SHP, cfg.W, cfg.NW, cfg.ncores
    GC, NB = cfg.GC, cfg.NBUCK

    deg = np.bincount(dst, minlength=N).astype(np.float64) + 1.0
    dinv = (1.0 / np.sqrt(deg)).astype(np.float32)

    # per-core edge lists (incl. self loops); cell = window * NB + bucket
    core_es, core_ed, core_cell = [], [], []
    NCELL = NW * NB
    counts = np.zeros((nc_, NCELL), np.int64)
    for c in range(nc_):
        lo, hi = c * SH, (c + 1) * SH
        m = (dst >= lo) & (dst < hi)
        es = np.concatenate([src[m], np.arange(lo, hi, dtype=np.int64)])
        ed = np.concatenate([dst[m] - lo, np.arange(SH, dtype=np.int64)])
        # table row id (shard-blocked)
        sh_i = es // SH
        loc = es - sh_i * SH
        gsl = sh_i * SHP + (loc % 128) * GC + (loc // 128)
        cell = (ed >> 6) * NB + (gsl >> 15)
        order = np.argsort(cell, kind="stable")
        core_es.append(gsl[order])
        core_ed.append(ed[order])
        core_cell.append(cell[order])
        counts[c] = np.bincount(cell, minlength=NCELL)

    ccell = (np.max(counts, axis=0) + 127) // 128     # chunks per cell
    # chunk order: [st][bucket][window in st]
    # cell (w, b) -> position: st = w//8; within st: bucket-major
    cell_k0 = np.zeros(NCELL, np.int64)               # first chunk of cell
    chunk_win, chunk_bucket = [], []
    st_ranges, calls = [], []
    k = 0
    for st in range(cfg.NST):
        k_st = k
        calls_st = []
        for b in range(NB):
            kb = k
            for w in range(st * 8, (st + 1) * 8):
                cell = w * NB + b
                cell_k0[cell] = k
                nch = int(ccell[cell])
                chunk_win.extend([w & 7] * nch)
                chunk_bucket.extend([b] * nch)
                k += nch
            if k > kb:
                calls_st.append((kb, k, b))
        st_ranges.append((k_st, k))
        calls.append(calls_st)
    K = k

    plan = {
        "chunk_win": chunk_win,
        "chunk_bucket": chunk_bucket,
        "st_ranges": st_ranges,
        "calls": calls,
        "K": K,
    }

    per_core = []
    for c in range(nc_):
        gsl, ed, cell = core_es[c], core_ed[c], core_cell[c]
        # rank of each edge within its cell
        c_off = np.zeros(NCELL + 1, np.int64)
        np.cumsum(counts[c], out=c_off[1:])
        rank = np.arange(len(ed)) - c_off[cell]
        slot = cell_k0[cell] * 128 + rank             # global slot id
        part = slot & 127
        col = slot >> 7

        seg_tab = np.full((128, K), 100.0, np.float32)
        seg_tab[part, col] = (ed & 63).astype(np.float32)

        # int16 idx stream: per supertile, slots renumbered from the ST's
        # first chunk; 16-wrap layout [16, n/16], replicated to 128 parts.
        idx_lin = np.zeros(K * 128, np.int16)
        idx_lin[slot] = (gsl - (gsl >> 15 << 15)).astype(np.int16)
        idx_tab = np.zeros((128, K * 8), np.int16)
        for st, (k0, k1) in enumerate(st_ranges):
            n = (k1 - k0) * 128
            blk = idx_lin[k0 * 128:k1 * 128].reshape(n // 16, 16).T
            idx_tab[:, k0 * 8:k1 * 8] = np.tile(blk, (8, 1))
        per_core.append({
            "idx_tab": idx_tab,
            "seg_tab": seg_tab.astype(BF16),
        })

    return dinv, plan, per_core


# ---------------------------------------------------------------- builder

def build_gcn(tc, io, cfg, plan):
    nc = tc.nc
    dt = mybir.dt
    Alu = mybir.AluOpType
    Act = mybir.ActivationFunctionType
    D = cfg.D
    SHP, NST, STW, GC = cfg.SHP, cfg.NST, cfg.STW, cfg.GC
    K = plan["K"]
    chunk_win = plan["chunk_win"]
    st_ranges = plan["st_ranges"]
    calls = plan["calls"]
    NL = 3
    CMAX = max(k1 - k0 for k0, k1 in st_ranges)

    with tc.tile_pool(name="res", bufs=1) as res, \
         tc.tile_pool(name="stp", bufs=2) as stp, \
         tc.tile_pool(name="pseg", bufs=2, space="PSUM") as pseg, \
         tc.tile_pool(name="pg", bufs=2, space="PSUM") as pg, \
         tc.tile_pool(name="dram", bufs=1, space="DRAM") as dram:

        # ---------------- resident tiles
        xt = res.tile([D, SHP], dt.bfloat16)
        y_t = res.tile([D, SHP], dt.bfloat16)
        dinvb = res.tile([D, SHP], dt.bfloat16)
        dinv_nm = res.tile([128, GC], dt.float32)
        seg_t = res.tile([128, K], dt.bfloat16)
        iota_t = res.tile([128, cfg.W], dt.bfloat16)
        zeros_t = res.tile([128, D], dt.bfloat16)
        junk_t = res.tile([128, STW], dt.bfloat16)
        gstage = res.tile([128, GC * D], dt.bfloat16)
        sums = res.tile([D, NST], dt.float32)
        sums2 = res.tile([D, NST], dt.float32)
        scratch = res.tile([D, STW], dt.bfloat16)
        stat_sb = res.tile([D, 2], dt.float32)
        W_t = [res.tile([D, D], dt.bfloat16, tag=f"W{i}", name=f"W{i}t")
               for i in range(NL)]
        b_t = [res.tile([D, 1], dt.float32, tag=f"b{i}", name=f"b{i}t")
               for i in range(NL)]
        gam_t = [res.tile([D, 1], dt.float32, tag=f"g{i}", name=f"g{i}t")
                 for i in range(NL)]
        bet_t = [res.tile([D, 1], dt.float32, tag=f"be{i}", name=f"be{i}t")
                 for i in range(NL)]
        mean_t = res.tile([D, 1], dt.float32)
        ex2_t = res.tile([D, 1], dt.float32)
        msq_t = res.tile([D, 1], dt.float32)
        var_t = res.tile([D, 1], dt.float32)
        sd_t = res.tile([D, 1], dt.float32)
        rinv_t = res.tile([D, 1], dt.float32)
        sc_t = res.tile([D, 1], dt.float32)
        tmp_t = res.tile([D, 1], dt.float32)
        sh_t = res.tile([D, 1], dt.float32)

        # ---------------- DRAM tiles
        g_own = dram.tile([SHP, BROW], dt.bfloat16)
        stat_in = dram.tile([D, 2], dt.float32)
        rg = [list(range(cfg.ncores))]

        # ---------------- load inputs / init
        nc.sync.dma_start(xt[:], io["xt0"][:])
        nc.sync.dma_start(dinvb[:], io["dinvb"][:])
        nc.sync.dma_start(dinv_nm[:], io["dinv_nm"][:])
        nc.sync.dma_start(seg_t[:], io["seg_tab"][:])
        for i in range(NL):
            nc.sync.dma_start(W_t[i][:], io[f"W{i+1}"][:])
            nc.sync.dma_start(b_t[i][:], io[f"b{i+1}"][:])
            nc.sync.dma_start(gam_t[i][:], io[f"gamma{i+1}"][:])
            nc.sync.dma_start(bet_t[i][:], io[f"beta{i+1}"][:])
        nc.gpsimd.iota(iota_t[:], pattern=[[1, cfg.W]], base=0,
                       channel_multiplier=0,
                       allow_small_or_imprecise_dtypes=True)
        nc.vector.memset(zeros_t[:], 0.0)
        nc.vector.memset(junk_t[:], 0.0)
        # NOTE: columns D..BROW of each g row are never written or read by
        # compute (the 256B row width only serves the gather engine's
        # stride constraint), so they are left uninitialized.

        inv_n = 1.0 / float(cfg.N)

        for L in range(NL):
            g_full = dram.tile([cfg.NTAB, BROW], dt.bfloat16,
                               addr_space="Shared", tag="g_full",
                               name=f"g_full{L}")
            stat_out = dram.tile([D, 2], dt.float32, addr_space="Shared",
                                 tag="stat_out", name=f"stat_out{L}")

            # ---- 1. g = dinv * (x @ W) -> DRAM (blocked rows, 256B wide)
            for c in range(GC):
                p = pg.tile([128, D], dt.float32, space="PSUM", tag="pg")
                nc.tensor.matmul(p[:], lhsT=xt[:, c * 128:(c + 1) * 128],
                                 rhs=W_t[L][:], start=True, stop=True)
                nc.scalar.activation(
                    gstage[:, c * D:(c + 1) * D], p[:],
                    Act.Identity, scale=dinv_nm[:, c:c + 1])
            nc.sync.dma_start(
                g_own[:].rearrange("(p c) r -> p c r", p=128)[:, :, 0:D],
                gstage[:].rearrange("p (c r) -> p c r", r=D))

            # ---- 2. AllGather g
            nc.gpsimd.collective_compute(
                "AllGather", Alu.bypass, replica_groups=rg,
                ins=[g_own[:]], outs=[g_full[:]])

            # ---- 3. supertile loop
            qn = 0
            for st in range(NST):
                k0, k1 = st_ranges[st]
                cst = k1 - k0
                idx_st = stp.tile([128, CMAX * 8], dt.int16, tag="idx")
                nc.sync.dma_start(idx_st[:, :cst * 8],
                                  io["idx_tab"][:, k0 * 8:k1 * 8])
                msgs = stp.tile([128, CMAX * BROW], dt.bfloat16, tag="msgs")
                CPC = 8      # chunks per gather call (num_idxs <= 1024)
                for (ck0, ck1, b) in calls[st]:
                    r0 = b * BUCK
                    r1 = min(r0 + BUCK, cfg.NTAB)
                    for q0 in range(ck0, ck1, CPC):
                        q1 = min(q0 + CPC, ck1)
                        nb = (q1 - q0) * 128
                        nc.gpsimd.dma_gather(
                            out_ap=msgs[:, (q0 - k0) * BROW:(q1 - k0) * BROW]
                                .rearrange("p (c r) -> p c r", r=BROW),
                            in_ap=g_full[r0:r1, :],
                            idxs_ap=idx_st[:, (q0 - k0) * 8:(q1 - k0) * 8],
                            num_idxs=nb,
                            num_idxs_reg=nb,
                            elem_size=BROW,
                            queue_num=qn & 3,
                        )
                        qn += 1

                oh = stp.tile([128, CMAX * cfg.W], dt.bfloat16, tag="oh")
                BB = 8
                for b0 in range(0, cst, BB):
                    b1 = min(b0 + BB, cst)
                    nbk = b1 - b0
                    nc.vector.tensor_tensor(
                        out=oh[:, b0 * cfg.W:b1 * cfg.W].rearrange(
                            "p (c w) -> p c w", w=cfg.W),
                        in0=seg_t[:, k0 + b0:k0 + b1].to_broadcast(
                            [128, nbk, cfg.W]),
                        in1=iota_t[:].rearrange("p (c w) -> p c w", c=1)
                            .to_broadcast([128, nbk, cfg.W]),
                        op=Alu.is_equal)

                ps = pseg.tile([D, STW], dt.float32, space="PSUM", tag="ps")
                nc.tensor.matmul(ps[:], lhsT=zeros_t[:], rhs=junk_t[:],
                                 start=True, stop=False, skip_group_check=True)
                for k in range(cst):
                    w = chunk_win[k0 + k]
                    nc.tensor.matmul(
                        ps[:, w * cfg.W:(w + 1) * cfg.W],
                        lhsT=msgs[:, k * BROW:k * BROW + D],
                        rhs=oh[:, k * cfg.W:(k + 1) * cfg.W],
                        start=False, stop=(k == cst - 1),
                        skip_group_check=True)

                tt = stp.tile([D, STW], dt.float32, tag="tt")
                nc.vector.tensor_tensor(
                    out=tt[:], in0=ps[:],
                    in1=dinvb[:, st * STW:(st + 1) * STW], op=Alu.mult)

                ysl = y_t[:, st * STW:(st + 1) * STW]
                if st < NST - 1 or cfg.SH == SHP:
                    nc.scalar.activation(ysl, tt[:], Act.Relu, bias=b_t[L][:],
                                         accum_out=sums[:, st:st + 1])
                    nc.scalar.activation(scratch[:], ysl, Act.Square,
                                         accum_out=sums2[:, st:st + 1])
                else:
                    nc.scalar.activation(ysl, tt[:], Act.Relu, bias=b_t[L][:])
                    V = cfg.SH - (NST - 1) * STW
                    yv = y_t[:, (NST - 1) * STW:(NST - 1) * STW + V]
                    nc.vector.reduce_sum(sums[:, st:st + 1], yv,
                                         axis=mybir.AxisListType.X)
                    nc.scalar.activation(scratch[:, :V], yv, Act.Square,
                                         accum_out=sums2[:, st:st + 1])

            # ---- 4. BN statistics + AllReduce
            nc.vector.reduce_sum(stat_sb[:, 0:1], sums[:],
                                 axis=mybir.AxisListType.X)
            nc.vector.reduce_sum(stat_sb[:, 1:2], sums2[:],
                                 axis=mybir.AxisListType.X)
            nc.sync.dma_start(stat_in[:], stat_sb[:])
            nc.gpsimd.collective_compute(
                "AllReduce", Alu.add, replica_groups=rg,
                ins=[stat_in[:]], outs=[stat_out[:]])
            nc.sync.dma_start(stat_sb[:], stat_out[:])

            nc.vector.tensor_scalar_mul(mean_t[:], stat_sb[:, 0:1], inv_n)
            nc.vector.tensor_scalar_mul(ex2_t[:], stat_sb[:, 1:2], inv_n)
            nc.vector.tensor_tensor(msq_t[:], in0=mean_t[:], in1=mean_t[:],
                                    op=Alu.mult)
            nc.vector.tensor_tensor(var_t[:], in0=ex2_t[:], in1=msq_t[:],
                                    op=Alu.subtract)
            nc.vector.tensor_scalar_add(var_t[:], var_t[:], cfg.eps)
            nc.scalar.activation(sd_t[:], var_t[:], Act.Sqrt, bias=0.0)
            nc.vector.reciprocal(rinv_t[:], sd_t[:])
            nc.vector.tensor_tensor(sc_t[:], in0=gam_t[L][:], in1=rinv_t[:],
                                    op=Alu.mult)
            nc.vector.tensor_tensor(tmp_t[:], in0=mean_t[:], in1=sc_t[:],
                                    op=Alu.mult)
            nc.vector.tensor_tensor(sh_t[:], in0=bet_t[L][:], in1=tmp_t[:],
                                    op=Alu.subtract)

            # ---- 5. BN affine
            QW = SHP // 4
            for q in range(4):
                if L == NL - 1:
                    ostg = stp.tile([D, QW], dt.float32, tag="ostg", bufs=1)
                    nc.scalar.activation(ostg[:], y_t[:, q * QW:(q + 1) * QW],
                                         Act.Identity, bias=sh_t[:],
                                         scale=sc_t[:])
                    nc.sync.dma_start(io["out_t"][:, q * QW:(q + 1) * QW],
                                      ostg[:])
                else:
                    nc.scalar.activation(xt[:, q * QW:(q + 1) * QW],
                                         y_t[:, q * QW:(q + 1) * QW],
                                         Act.Identity, bias=sh_t[:],
                                         scale=sc_t[:])


# ---------------------------------------------------------------- runner

def _ensure_ntff_hook():
    """Provide antenv.axon_hooks on images where it's missing, so
    run_bass_kernel_spmd(trace=True) can capture NTFF profiles."""
    import sys
    import types
    try:
        from antenv.axon_hooks import get_axon_ntff_profile_hook  # noqa: F401
        return
    except ImportError:
        pass
    try:
        import antenv
        from trn_agent_boot.trn_boot import _ntff_profile_via_ctypes
        hook = _ntff_profile_via_ctypes('/opt/axon/libaxon_pjrt.so')
    except Exception:
        hook = None
        import antenv
    mod = types.ModuleType('antenv.axon_hooks')
    mod.get_axon_ntff_profile_hook = lambda: hook
    mod.set_axon_ntff_profile_hook = lambda h: None
    sys.modules['antenv.axon_hooks'] = mod
    antenv.axon_hooks = mod
    bass_utils.upload_artifacts = lambda d: "local://" + d


def _run(cfg, inputs, trace=False):
    if trace:
        _ensure_ntff_hook()
    edge_index = inputs["edge_index"]
    node_attr = np.asarray(inputs["node_attr"], dtype=np.float32)

    dinv, plan, per_core = preprocess(cfg, edge_index)

    nc = bacc.Bacc("TRN2", target_bir_lowering=False, debug=False,
                   enable_asserts=False, num_devices=cfg.ncores,
                   num_swdge_queues=4)

    D, SHP, GC, K = cfg.D, cfg.SHP, cfg.GC, plan["K"]
    io = {}

    def dram_in(name, shape, d):
        io[name] = nc.dram_tensor(name, shape, d, kind="ExternalInput").ap()

    dram_in("xt0", [D, SHP], mybir.dt.bfloat16)
    dram_in("dinvb", [D, SHP], mybir.dt.bfloat16)
    dram_in("dinv_nm", [128, GC], mybir.dt.float32)
    dram_in("idx_tab", [128, K * 8], mybir.dt.int16)
    dram_in("seg_tab", [128, K], mybir.dt.bfloat16)
    for i in (1, 2, 3):
        dram_in(f"W{i}", [D, D], mybir.dt.bfloat16)
        dram_in(f"b{i}", [D, 1], mybir.dt.float32)
        dram_in(f"gamma{i}", [D, 1], mybir.dt.float32)
        dram_in(f"beta{i}", [D, 1], mybir.dt.float32)
    io["out_t"] = nc.dram_tensor("out_t", [D, SHP], mybir.dt.float32,
                                 kind="ExternalOutput").ap()

    with tile.TileContext(nc) as tc:
        build_gcn(tc, io, cfg, plan)
    nc.compile()

    in_maps = []
    for c in range(cfg.ncores):
        lo = c * cfg.SH
        xs = np.zeros((SHP, D), np.float32)
        xs[:cfg.SH] = node_attr[lo:lo + cfg.SH]
        dv = np.zeros(SHP, np.float32)
        dv[:cfg.SH] = dinv[lo:lo + cfg.SH]
        dnm = np.zeros((128, GC), np.float32)
        loc = np.arange(cfg.SH)
        dnm[loc % 128, loc // 128] = dinv[lo:lo + cfg.SH]
        m = {
            "xt0": np.ascontiguousarray(xs.T).astype(BF16),
            "dinvb": np.broadcast_to(dv.astype(BF16), (D, SHP)).copy(),
            "dinv_nm": dnm,
            "idx_tab": per_core[c]["idx_tab"],
            "seg_tab": per_core[c]["seg_tab"],
        }
        for i in (1, 2, 3):
            m[f"W{i}"] = np.asarray(inputs[f"W{i}"], np.float32).astype(BF16)
            m[f"b{i}"] = np.asarray(
                inputs[f"b{i}"], np.float32).reshape(D, 1)
            m[f"gamma{i}"] = np.asarray(
                inputs[f"gamma{i}"], np.float32).reshape(D, 1)
            m[f"beta{i}"] = np.asarray(
                inputs[f"beta{i}"], np.float32).reshape(D, 1)
        in_maps.append(m)

    res = bass_utils.run_bass_kernel_spmd(
        nc, in_maps, core_ids=list(range(cfg.ncores)), trace=trace)

    out = np.empty((cfg.N, D), np.float32)
    for c in range(cfg.ncores):
        ot = res.results[c]["out_t"]          # [D, SHP]
        out[c * cfg.SH:(c + 1) * cfg.SH] = ot[:, :cfg.SH].T
    return out, res


def kernel(**inputs):
    trace = bool(int(os.environ.get("GCN_TRACE", "0")))
    out, res = _run(REAL, inputs, trace=trace)
    if trace and res.exec_time_ns is not None:
        print(f"HW exec time: {res.exec_time_ns} ns")
        kernel.last_exec_ns = res.exec_time_ns
    kernel.last_results = res
    return out



# revision 2
# speedup vs baseline: 3.4432x; 3.4432x over previous
"""GCN encoder (3x [GCNConv -> ReLU -> BatchNorm]) on 8 Trainium2 NeuronCores.

Strategy (graph/data parallel, dst-sharded):
  - Nodes sharded 8 ways by dst; each core owns its node shard and all edges
    whose dst lands in the shard. Self-loops are NOT materialized as edges:
    the self contribution dinv^2*(xW) is injected as the PSUM-initializing
    matmul of each supertile (lhsT=W, rhs=xtd slice, start=True).
  - Per layer: each core computes g = (x*dinv) @ W for its shard (PE),
    AllGather g -> full bf16 table in DRAM (rows padded to 256 B for the
    gather engine), then dma_gather pulls g[src] rows for its edges,
    the TensorEngine segment-sums them using on-the-fly one-hot matrices
    (DVE iota-compare, window=256 dsts), then dinv/bias/ReLU, BN statistics
    (free-axis reduce; feature-major layout), AllReduce of the 64x2 stats,
    BN affine fused with the next layer's dinv scaling.
  - Feature data is bf16; accumulation f32 in PSUM; statistics f32.

The gather uses the custom InstDMAGatherAnt (int16 indices, 256B-multiple
row stride), so sources are bucketed by table-row>>15 into 4 index buckets;
chunks of 128 edges are (st, bucket, window)-pure. Chunk counts per cell are
equalized across cores (SPMD: one program, per-core data).

Host-side preprocessing (numpy, off the measured HW path): degree/dinv,
edge sharding + cell packing, index tables, final unshard/transpose.
"""

import os
import numpy as np
import ml_dtypes

import concourse.bass as bass
import concourse.bacc as bacc
import concourse.mybir as mybir
import concourse.tile as tile
from concourse import bass_utils

BF16 = ml_dtypes.bfloat16
BROW = 128          # table row width (elements) -> 256 B in bf16
BUCK = 1 << 15      # index-bucket size (int16 positive range)

# ---------------------------------------------------------------- config

class Cfg:
    def __init__(self, N, E, D=64, ncores=8, eps=1e-5):
        self.N = N
        self.E = E
        self.D = D
        self.ncores = ncores
        self.eps = eps
        self.SH = N // ncores
        assert self.SH * ncores == N
        self.W = 256             # one-hot window width (dst grid)
        self.NWST = 2            # windows per supertile (512/W)
        self.STW = 512           # dsts per supertile (one PSUM bank)
        self.SHP = ((self.SH + 511) // 512) * 512
        self.NST = self.SHP // self.STW
        self.GC = self.SHP // 128
        self.NTAB = ncores * self.SHP
        self.NBUCK = (self.NTAB + BUCK - 1) // BUCK


REAL = Cfg(N=100000, E=1600000)

# ---------------------------------------------------------------- host prep

def preprocess(cfg, edge_index):
    """Build per-core gather/segment tables + the shared compile-time plan.

    Chunk order: [supertile][bucket][window-within-st]; each chunk is 128
    edge slots, (st, bucket, window)-pure. Gather call (st, b) covers that
    bucket's contiguous chunk range of the supertile.

    Returns (dinv, plan, per_core) where plan has:
      chunk_win[k]   window-within-st (0..NWST-1) of chunk k
      st_ranges[st]  (k0, k1) chunk range of supertile st
      calls[st]      list of (k0, k1, bucket) gather calls
    """
    src = np.asarray(edge_index[0], dtype=np.int64)
    dst = np.asarray(edge_index[1], dtype=np.int64)
    N, SH, SHP, W, nc_ = cfg.N, cfg.SH, cfg.SHP, cfg.W, cfg.ncores
    GC, NB, NWST = cfg.GC, cfg.NBUCK, cfg.NWST

    deg = np.bincount(dst, minlength=N).astype(np.float64) + 1.0
    dinv = (1.0 / np.sqrt(deg)).astype(np.float32)

    # per-core edge lists (NO self loops); cell = (st*NWST + win) * NB + bucket
    NWIN = cfg.NST * NWST
    NCELL = NWIN * NB
    core_es, core_ed, core_cell = [], [], []
    counts = np.zeros((nc_, NCELL), np.int64)
    for c in range(nc_):
        lo, hi = c * SH, (c + 1) * SH
        m = (dst >= lo) & (dst < hi)
        es = src[m]
        ed = dst[m] - lo
        # table row id (shard-blocked)
        sh_i = es // SH
        loc = es - sh_i * SH
        gsl = sh_i * SHP + (loc % 128) * GC + (loc // 128)
        cell = (ed // W) * NB + (gsl >> 15)
        order = np.argsort(cell, kind="stable")
        core_es.append(gsl[order])
        core_ed.append(ed[order])
        core_cell.append(cell[order])
        counts[c] = np.bincount(cell, minlength=NCELL)

    ccell = (np.max(counts, axis=0) + 127) // 128     # chunks per cell
    np.maximum(ccell, 1, out=ccell)                   # keep >= 1 for safety
    # chunk order: [st][bucket][window in st]
    cell_k0 = np.zeros(NCELL, np.int64)               # first chunk of cell
    chunk_win = []
    st_ranges, calls = [], []
    k = 0
    for st in range(cfg.NST):
        k_st = k
        calls_st = []
        for b in range(NB):
            kb = k
            for w in range(st * NWST, (st + 1) * NWST):
                cell = w * NB + b
                cell_k0[cell] = k
                nch = int(ccell[cell])
                chunk_win.extend([w % NWST] * nch)
                k += nch
            if k > kb:
                calls_st.append((kb, k, b))
        st_ranges.append((k_st, k))
        calls.append(calls_st)
    K = k

    plan = {
        "chunk_win": chunk_win,
        "st_ranges": st_ranges,
        "calls": calls,
        "K": K,
    }

    per_core = []
    for c in range(nc_):
        gsl, ed, cell = core_es[c], core_ed[c], core_cell[c]
        # rank of each edge within its cell
        c_off = np.zeros(NCELL + 1, np.int64)
        np.cumsum(counts[c], out=c_off[1:])
        rank = np.arange(len(ed)) - c_off[cell]
        slot = cell_k0[cell] * 128 + rank             # global slot id
        part = slot & 127
        col = slot >> 7

        seg_tab = np.full((128, K), 300.0, np.float32)
        seg_tab[part, col] = (ed % W).astype(np.float32)

        # int16 idx stream: 16-wrap layout [16, n/16] per supertile,
        # replicated to 128 parts.
        idx_lin = np.zeros(K * 128, np.int16)
        idx_lin[slot] = (gsl - (gsl >> 15 << 15)).astype(np.int16)
        idx_tab = np.zeros((128, K * 8), np.int16)
        for st, (k0, k1) in enumerate(st_ranges):
            n = (k1 - k0) * 128
            blk = idx_lin[k0 * 128:k1 * 128].reshape(n // 16, 16).T
            idx_tab[:, k0 * 8:k1 * 8] = np.tile(blk, (8, 1))
        per_core.append({
            "idx_tab": idx_tab,
            "seg_tab": seg_tab.astype(BF16),
        })

    return dinv, plan, per_core


# ---------------------------------------------------------------- builder

def build_gcn(tc, io, cfg, plan):
    nc = tc.nc
    dt = mybir.dt
    Alu = mybir.AluOpType
    Act = mybir.ActivationFunctionType
    D = cfg.D
    SHP, NST, STW, GC, W = cfg.SHP, cfg.NST, cfg.STW, cfg.GC, cfg.W
    K = plan["K"]
    chunk_win = plan["chunk_win"]
    st_ranges = plan["st_ranges"]
    calls = plan["calls"]
    NL = 3
    CMAX = max(k1 - k0 for k0, k1 in st_ranges)

    with tc.tile_pool(name="res", bufs=1) as res, \
         tc.tile_pool(name="stp", bufs=3) as stp, \
         tc.tile_pool(name="ohp", bufs=2) as ohp, \
         tc.tile_pool(name="pseg", bufs=2, space="PSUM") as pseg, \
         tc.tile_pool(name="pg", bufs=2, space="PSUM") as pg, \
         tc.tile_pool(name="dram", bufs=1, space="DRAM") as dram:

        # ---------------- resident tiles
        xtd = res.tile([D, SHP], dt.bfloat16)       # dinv-scaled features
        y_t = res.tile([D, SHP], dt.bfloat16)
        xnew = res.tile([D, SHP], dt.bfloat16)      # BN affine output
        dinvb = res.tile([D, SHP], dt.bfloat16)
        seg_t = res.tile([128, K], dt.bfloat16)
        iota_t = res.tile([128, W], dt.bfloat16)
        gstage = res.tile([128, GC * D], dt.bfloat16)
        sums = res.tile([D, NST], dt.float32)
        sums2 = res.tile([D, NST], dt.float32)
        scratch = res.tile([D, STW], dt.bfloat16)
        stat_sb = res.tile([D, 2], dt.float32)
        W_t = [res.tile([D, D], dt.bfloat16, tag=f"W{i}", name=f"W{i}t")
               for i in range(NL)]
        b_t = [res.tile([D, 1], dt.float32, tag=f"b{i}", name=f"b{i}t")
               for i in range(NL)]
        gam_t = [res.tile([D, 1], dt.float32, tag=f"g{i}", name=f"g{i}t")
                 for i in range(NL)]
        bet_t = [res.tile([D, 1], dt.float32, tag=f"be{i}", name=f"be{i}t")
                 for i in range(NL)]
        mean_t = res.tile([D, 1], dt.float32)
        ex2_t = res.tile([D, 1], dt.float32)
        msq_t = res.tile([D, 1], dt.float32)
        var_t = res.tile([D, 1], dt.float32)
        sd_t = res.tile([D, 1], dt.float32)
        rinv_t = res.tile([D, 1], dt.float32)
        sc_t = res.tile([D, 1], dt.float32)
        tmp_t = res.tile([D, 1], dt.float32)
        sh_t = res.tile([D, 1], dt.float32)

        # ---------------- DRAM tiles
        g_own = dram.tile([SHP, BROW], dt.bfloat16)
        stat_in = dram.tile([D, 2], dt.float32)
        rg = [list(range(cfg.ncores))]

        # ---------------- load inputs / init
        nc.sync.dma_start(xtd[:], io["xtd0"][:])
        nc.sync.dma_start(dinvb[:], io["dinvb"][:])
        nc.sync.dma_start(seg_t[:], io["seg_tab"][:])
        for i in range(NL):
            nc.sync.dma_start(W_t[i][:], io[f"W{i+1}"][:])
            nc.sync.dma_start(b_t[i][:], io[f"b{i+1}"][:])
            nc.sync.dma_start(gam_t[i][:], io[f"gamma{i+1}"][:])
            nc.sync.dma_start(bet_t[i][:], io[f"beta{i+1}"][:])
        nc.gpsimd.iota(iota_t[:], pattern=[[1, W]], base=0,
                       channel_multiplier=0,
                       allow_small_or_imprecise_dtypes=True)
        # NOTE: columns D..BROW of each g row are never written or read by
        # compute (the 256B row width only serves the gather engine's
        # stride constraint), so they are left uninitialized.

        inv_n = 1.0 / float(cfg.N)

        for L in range(NL):
            g_full = dram.tile([cfg.NTAB, BROW], dt.bfloat16,
                               addr_space="Shared", tag="g_full",
                               name=f"g_full{L}")
            stat_out = dram.tile([D, 2], dt.float32, addr_space="Shared",
                                 tag="stat_out", name=f"stat_out{L}")

            # ---- 1. g = (x*dinv) @ W -> DRAM (blocked rows, 256B wide)
            for c in range(GC):
                p = pg.tile([128, D], dt.float32, space="PSUM", tag="pg")
                nc.tensor.matmul(p[:], lhsT=xtd[:, c * 128:(c + 1) * 128],
                                 rhs=W_t[L][:], start=True, stop=True)
                nc.scalar.activation(
                    gstage[:, c * D:(c + 1) * D], p[:], Act.Identity)
            nc.sync.dma_start(
                g_own[:].rearrange("(p c) r -> p c r", p=128)[:, :, 0:D],
                gstage[:].rearrange("p (c r) -> p c r", r=D))

            # ---- 2. AllGather g
            nc.gpsimd.collective_compute(
                "AllGather", Alu.bypass, replica_groups=rg,
                ins=[g_own[:]], outs=[g_full[:]])

            # ---- 3. supertile loop
            qn = 0
            for st in range(NST):
                k0, k1 = st_ranges[st]
                cst = k1 - k0
                idx_st = stp.tile([128, CMAX * 8], dt.int16, tag="idx")
                nc.sync.dma_start(idx_st[:, :cst * 8],
                                  io["idx_tab"][:, k0 * 8:k1 * 8])
                msgs = stp.tile([128, CMAX * BROW], dt.bfloat16, tag="msgs")
                CPC = 8      # chunks per gather call (num_idxs <= 1024)
                for (ck0, ck1, b) in calls[st]:
                    r0 = b * BUCK
                    r1 = min(r0 + BUCK, cfg.NTAB)
                    for q0 in range(ck0, ck1, CPC):
                        q1 = min(q0 + CPC, ck1)
                        nb = (q1 - q0) * 128
                        nc.gpsimd.dma_gather(
                            out_ap=msgs[:, (q0 - k0) * BROW:(q1 - k0) * BROW]
                                .rearrange("p (c r) -> p c r", r=BROW),
                            in_ap=g_full[r0:r1, :],
                            idxs_ap=idx_st[:, (q0 - k0) * 8:(q1 - k0) * 8],
                            num_idxs=nb,
                            num_idxs_reg=nb,
                            elem_size=BROW,
                            queue_num=qn & 3,
                        )
                        qn += 1

                # PSUM init: self-loop term dinv_d * (x*dinv W)[d] = rhs xtd
                ps = pseg.tile([D, STW], dt.float32, space="PSUM", tag="ps")
                nc.tensor.matmul(ps[:], lhsT=W_t[L][:],
                                 rhs=xtd[:, st * STW:(st + 1) * STW],
                                 start=True, stop=False, skip_group_check=True)

                BB = 8
                for b0 in range(0, cst, BB):
                    b1 = min(b0 + BB, cst)
                    nbk = b1 - b0
                    oh = ohp.tile([128, BB * W], dt.bfloat16, tag="oh")
                    nc.vector.tensor_tensor(
                        out=oh[:, :nbk * W].rearrange(
                            "p (c w) -> p c w", w=W),
                        in0=seg_t[:, k0 + b0:k0 + b1].to_broadcast(
                            [128, nbk, W]),
                        in1=iota_t[:].rearrange("p (c w) -> p c w", c=1)
                            .to_broadcast([128, nbk, W]),
                        op=Alu.is_equal)
                    for k in range(b0, b1):
                        w = chunk_win[k0 + k]
                        nc.tensor.matmul(
                            ps[:, w * W:(w + 1) * W],
                            lhsT=msgs[:, k * BROW:k * BROW + D],
                            rhs=oh[:, (k - b0) * W:(k - b0 + 1) * W],
                            start=False, stop=(k == cst - 1),
                            skip_group_check=True)

                tt = stp.tile([D, STW], dt.float32, tag="tt")
                nc.vector.tensor_tensor(
                    out=tt[:], in0=ps[:],
                    in1=dinvb[:, st * STW:(st + 1) * STW], op=Alu.mult)

                ysl = y_t[:, st * STW:(st + 1) * STW]
                if st < NST - 1 or cfg.SH == SHP:
                    nc.scalar.activation(ysl, tt[:], Act.Relu, bias=b_t[L][:],
                                         accum_out=sums[:, st:st + 1])
                    nc.scalar.activation(scratch[:], ysl, Act.Square,
                                         accum_out=sums2[:, st:st + 1])
                else:
                    nc.scalar.activation(ysl, tt[:], Act.Relu, bias=b_t[L][:])
                    V = cfg.SH - (NST - 1) * STW
                    yv = y_t[:, (NST - 1) * STW:(NST - 1) * STW + V]
                    nc.vector.reduce_sum(sums[:, st:st + 1], yv,
                                         axis=mybir.AxisListType.X)
                    nc.scalar.activation(scratch[:, :V], yv, Act.Square,
                                         accum_out=sums2[:, st:st + 1])

            # ---- 4. BN statistics + AllReduce
            nc.vector.reduce_sum(stat_sb[:, 0:1], sums[:],
                                 axis=mybir.AxisListType.X)
            nc.vector.reduce_sum(stat_sb[:, 1:2], sums2[:],
                                 axis=mybir.AxisListType.X)
            nc.sync.dma_start(stat_in[:], stat_sb[:])
            nc.gpsimd.collective_compute(
                "AllReduce", Alu.add, replica_groups=rg,
                ins=[stat_in[:]], outs=[stat_out[:]])
            nc.sync.dma_start(stat_sb[:], stat_out[:])

            nc.vector.tensor_scalar_mul(mean_t[:], stat_sb[:, 0:1], inv_n)
            nc.vector.tensor_scalar_mul(ex2_t[:], stat_sb[:, 1:2], inv_n)
            nc.vector.tensor_tensor(msq_t[:], in0=mean_t[:], in1=mean_t[:],
                                    op=Alu.mult)
            nc.vector.tensor_tensor(var_t[:], in0=ex2_t[:], in1=msq_t[:],
                                    op=Alu.subtract)
            nc.vector.tensor_scalar_add(var_t[:], var_t[:], cfg.eps)
            nc.scalar.activation(sd_t[:], var_t[:], Act.Sqrt, bias=0.0)
            nc.vector.reciprocal(rinv_t[:], sd_t[:])
            nc.vector.tensor_tensor(sc_t[:], in0=gam_t[L][:], in1=rinv_t[:],
                                    op=Alu.mult)
            nc.vector.tensor_tensor(tmp_t[:], in0=mean_t[:], in1=sc_t[:],
                                    op=Alu.mult)
            nc.vector.tensor_tensor(sh_t[:], in0=bet_t[L][:], in1=tmp_t[:],
                                    op=Alu.subtract)

            # ---- 5. BN affine (+ dinv fold for the next layer's xtd)
            QW = SHP // 4
            for q in range(4):
                if L == NL - 1:
                    ostg = stp.tile([D, QW], dt.float32, tag="ostg", bufs=1)
                    nc.scalar.activation(ostg[:], y_t[:, q * QW:(q + 1) * QW],
                                         Act.Identity, bias=sh_t[:],
                                         scale=sc_t[:])
                    nc.sync.dma_start(io["out_t"][:, q * QW:(q + 1) * QW],
                                      ostg[:])
                else:
                    nc.scalar.activation(xnew[:, q * QW:(q + 1) * QW],
                                         y_t[:, q * QW:(q + 1) * QW],
                                         Act.Identity, bias=sh_t[:],
                                         scale=sc_t[:])
                    nc.vector.tensor_tensor(
                        out=xtd[:, q * QW:(q + 1) * QW],
                        in0=xnew[:, q * QW:(q + 1) * QW],
                        in1=dinvb[:, q * QW:(q + 1) * QW], op=Alu.mult)


# ---------------------------------------------------------------- runner

def _ensure_ntff_hook():
    """Provide antenv.axon_hooks on images where it's missing, so
    run_bass_kernel_spmd(trace=True) can capture NTFF profiles."""
    import sys
    import types
    try:
        from antenv.axon_hooks import get_axon_ntff_profile_hook  # noqa: F401
        return
    except ImportError:
        pass
    try:
        import antenv
        from trn_agent_boot.trn_boot import _ntff_profile_via_ctypes
        hook = _ntff_profile_via_ctypes('/opt/axon/libaxon_pjrt.so')
    except Exception:
        hook = None
        import antenv
    mod = types.ModuleType('antenv.axon_hooks')
    mod.get_axon_ntff_profile_hook = lambda: hook
    mod.set_axon_ntff_profile_hook = lambda h: None
    sys.modules['antenv.axon_hooks'] = mod
    antenv.axon_hooks = mod
    bass_utils.upload_artifacts = lambda d: "local://" + d


def _run(cfg, inputs, trace=False):
    if trace:
        _ensure_ntff_hook()
    edge_index = inputs["edge_index"]
    node_attr = np.asarray(inputs["node_attr"], dtype=np.float32)

    dinv, plan, per_core = preprocess(cfg, edge_index)

    nc = bacc.Bacc("TRN2", target_bir_lowering=False, debug=False,
                   enable_asserts=False, num_devices=cfg.ncores,
                   num_swdge_queues=4)

    D, SHP, GC, K = cfg.D, cfg.SHP, cfg.GC, plan["K"]
    io = {}

    def dram_in(name, shape, d):
        io[name] = nc.dram_tensor(name, shape, d, kind="ExternalInput").ap()

    dram_in("xtd0", [D, SHP], mybir.dt.bfloat16)
    dram_in("dinvb", [D, SHP], mybir.dt.bfloat16)
    dram_in("idx_tab", [128, K * 8], mybir.dt.int16)
    dram_in("seg_tab", [128, K], mybir.dt.bfloat16)
    for i in (1, 2, 3):
        dram_in(f"W{i}", [D, D], mybir.dt.bfloat16)
        dram_in(f"b{i}", [D, 1], mybir.dt.float32)
        dram_in(f"gamma{i}", [D, 1], mybir.dt.float32)
        dram_in(f"beta{i}", [D, 1], mybir.dt.float32)
    io["out_t"] = nc.dram_tensor("out_t", [D, SHP], mybir.dt.float32,
                                 kind="ExternalOutput").ap()

    with tile.TileContext(nc) as tc:
        build_gcn(tc, io, cfg, plan)
    nc.compile()

    in_maps = []
    for c in range(cfg.ncores):
        lo = c * cfg.SH
        xs = np.zeros((SHP, D), np.float32)
        xs[:cfg.SH] = node_attr[lo:lo + cfg.SH]
        dv = np.zeros(SHP, np.float32)
        dv[:cfg.SH] = dinv[lo:lo + cfg.SH]
        xsd = xs * dv[:, None]
        m = {
            "xtd0": np.ascontiguousarray(xsd.T).astype(BF16),
            "dinvb": np.broadcast_to(dv.astype(BF16), (D, SHP)).copy(),
            "idx_tab": per_core[c]["idx_tab"],
            "seg_tab": per_core[c]["seg_tab"],
        }
        for i in (1, 2, 3):
            m[f"W{i}"] = np.asarray(inputs[f"W{i}"], np.float32).astype(BF16)
            m[f"b{i}"] = np.asarray(
                inputs[f"b{i}"], np.float32).reshape(D, 1)
            m[f"gamma{i}"] = np.asarray(
                inputs[f"gamma{i}"], np.float32).reshape(D, 1)
            m[f"beta{i}"] = np.asarray(
                inputs[f"beta{i}"], np.float32).reshape(D, 1)
        in_maps.append(m)

    res = bass_utils.run_bass_kernel_spmd(
        nc, in_maps, core_ids=list(range(cfg.ncores)), trace=trace)

    out = np.empty((cfg.N, D), np.float32)
    for c in range(cfg.ncores):
        ot = res.results[c]["out_t"]          # [D, SHP]
        out[c * cfg.SH:(c + 1) * cfg.SH] = ot[:, :cfg.SH].T
    return out, res


def kernel(**inputs):
    trace = bool(int(os.environ.get("GCN_TRACE", "0")))
    out, res = _run(REAL, inputs, trace=trace)
    if trace and res.exec_time_ns is not None:
        print(f"HW exec time: {res.exec_time_ns} ns")
        kernel.last_exec_ns = res.exec_time_ns
    kernel.last_results = res
    return out
